# revision 7
# baseline (speedup 1.0000x reference)
"""GatedAttention TRN2 kernel — 8-core tensor-parallel, fused-pipeline v2.

Self-contained: host-side shard/layout prep + Bass/Tile kernel + gather.
One kv-head group (4 q heads) per core; host sums the 8 output partials.

v2 restructure vs v1: the per-block phases (QKV projection, attention,
output projection) no longer run as separate engine-serial stretches.
Instead, attention(B)'s J-loop is the spine, and QKV(B+1) / outproj(B-1)
matmuls are emitted as fine-grained filler between J iterations, so the
PE streams continuously while ACT drains softmax exps:
  - QKV runs as 4 per-cc accumulation chains (16 hc MMs each) into a
    single rotating PSUM bank; each chain is extracted immediately
    (q -> DVE, k/v -> ACT, gate -> one batched ACT exp over the whole
    [128,512] tile instead of 4 single-row exps).
  - softmax exps are hh-batched: both heads of a (p,J) pair land in one
    [128,2,512] two-bank PSUM tile and one ACT exp (FD up to 1024) with
    the shared per-partition k-rms scale.
  - q-side RMS Newton runs once per block on [128,1024] tiles (both p
    halves) with in-place DVE chains.
  - PSUM budget (8 banks): scores/transients [128,1024]x2 (4) +
    ps_att [128,2,512] (2) + qkv chain (1) + outproj (1).
  - bulk DMA descriptors batched ([128,2,512] x-pairs, [128,1024] out
    stores) to cut gpsimd SWDGE descriptor time; first W/x chunks ride
    the 2 HWDGE queues (sync/scalar) which start ~3us earlier.
Engine-SBUF access patterns must start at partitions 0/32/64/96; PSUM
row slices are exempt. Cross-partition copies are DVE-only.
"""
import math
import os
import sys
import numpy as np
import ml_dtypes

BF16 = ml_dtypes.bfloat16

H, NH, KVH, HD = 2048, 32, 8, 64
G = NH // KVH          # 4 q heads per core
S = 2048
EPS = 1e-6
THETA = 1000000.0
SCALE = 1.0 / math.sqrt(HD)
NCORES = 8
HC = H // 128          # 16 h-chunks
NB = S // 512          # 4 si-blocks
NJ = S // 128          # 16 sj-chunks

_BUILT = {}
LAST_EXEC_NS = None


# ---------------------------------------------------------------- host prep
def _host_prep(hidden_states, Wq, Wk, Wv, Wo, g_q, g_k):
    x = np.ascontiguousarray(np.asarray(hidden_states, np.float32).reshape(S, H))
    Wq = np.asarray(Wq, np.float32)
    Wk = np.asarray(Wk, np.float32)
    Wv = np.asarray(Wv, np.float32)
    Wo = np.asarray(Wo, np.float32)
    g_q = np.asarray(g_q, np.float32)
    g_k = np.asarray(g_k, np.float32)

    xT = np.ascontiguousarray(x.T).astype(BF16)

    inv_freq = 1.0 / (THETA ** (np.arange(0, HD, 2, dtype=np.float32) / HD))
    pos = np.arange(S, dtype=np.float32)
    emb = np.concatenate([pos[:, None] * inv_freq[None, :]] * 2, axis=-1)  # [S,64]
    cos = np.cos(emb).T.astype(np.float32)   # [64, S]
    sin = np.sin(emb).T.astype(np.float32)
    sign = np.where(np.arange(HD) < HD // 2, -1.0, 1.0).astype(np.float32)[:, None]
    cosq = np.ascontiguousarray(cos * g_q[:, None]).astype(BF16)
    sinq = np.ascontiguousarray(sin * sign * np.roll(g_q, -32)[:, None]).astype(BF16)
    cosk = np.ascontiguousarray(cos * g_k[:, None]).astype(BF16)
    sink = np.ascontiguousarray(sin * sign * np.roll(g_k, -32)[:, None]).astype(BF16)

    in_maps = []
    for c in range(NCORES):
        Wq_g = Wq[:, c * (G * HD + G):(c + 1) * (G * HD + G)]
        gpad = np.zeros((H, 128), np.float32)
        for p in range(2):
            for hh in range(2):
                # gate for head (p,hh) lands on PSUM partition 64p+32hh — a
                # legal SBUF partition start for the sig_q reads downstream
                gpad[:, 64 * p + 32 * hh] = Wq_g[:, G * HD + 2 * p + hh]
        W_c = np.ascontiguousarray(np.concatenate(
            [Wq_g[:, :G * HD],
             Wk[:, c * HD:(c + 1) * HD],
             Wv[:, c * HD:(c + 1) * HD],
             gpad], axis=1))                                   # [H, 512]
        Wo_c = np.ascontiguousarray(Wo[c * G * HD:(c + 1) * G * HD, :])  # [256,H]
        in_maps.append({"xT": xT, "W": W_c.astype(BF16), "Wo": Wo_c.astype(BF16),
                        "cosq": cosq, "sinq": sinq, "cosk": cosk, "sink": sink})
    return in_maps


# ---------------------------------------------------------------- bass build
def _build_nc():
    import concourse.bass as bass
    import concourse.mybir as mybir
    import concourse.tile as tile
    from concourse import bacc
    from concourse.masks import make_identity, make_upper_triangular

    dt = mybir.dt
    f32 = dt.float32
    bf16 = dt.bfloat16
    u32 = dt.uint32
    AF = mybir.ActivationFunctionType
    MUL = mybir.AluOpType.mult
    ADD = mybir.AluOpType.add

    nc = bacc.Bacc("TRN2", target_bir_lowering=False, debug=False,
                   num_devices=NCORES)

    xT_d = nc.dram_tensor("xT", [H, S], bf16, kind="ExternalInput")
    W_d = nc.dram_tensor("W", [H, 512], bf16, kind="ExternalInput")
    Wo_d = nc.dram_tensor("Wo", [G * HD, H], bf16, kind="ExternalInput")
    cosq_d = nc.dram_tensor("cosq", [HD, S], bf16, kind="ExternalInput")
    sinq_d = nc.dram_tensor("sinq", [HD, S], bf16, kind="ExternalInput")
    cosk_d = nc.dram_tensor("cosk", [HD, S], bf16, kind="ExternalInput")
    sink_d = nc.dram_tensor("sink", [HD, S], bf16, kind="ExternalInput")
    out_d = nc.dram_tensor("out", [S, H], f32, kind="ExternalOutput")

    xT_r = xT_d.ap().rearrange("(c p) s -> p c s", p=128)    # [128,16,S]
    W_r = W_d.ap().rearrange("(c p) w -> p c w", p=128)      # [128,16,512]

    SIGMA = 0.0430
    EXPBIT_SCALE = math.log(2.0) / (1 << 23)

    import contextlib
    with tile.TileContext(nc) as tc, contextlib.ExitStack() as ctx:
        const = ctx.enter_context(tc.tile_pool(name="const", bufs=1))
        big = ctx.enter_context(tc.tile_pool(name="big", bufs=1))
        xpool = ctx.enter_context(tc.tile_pool(name="xp", bufs=16))
        wkp = ctx.enter_context(tc.tile_pool(name="wkp", bufs=2))
        psum = ctx.enter_context(tc.tile_pool(name="ps", bufs=1, space="PSUM"))

        # ---------------- constants
        id64 = const.tile([64, 64], bf16, tag="id64")
        tri = const.tile([128, 128], bf16, tag="tri")
        ones = const.tile([128, 1], bf16, tag="ones")
        nc.vector.memset(ones, 1.0)
        # block-diagonal selector: sums 64-row head blocks AND broadcasts the
        # sum back to all 64 rows of the head
        esel2 = const.tile([128, 128], bf16, tag="esel2")
        nc.vector.memset(esel2, 0.0)
        nc.vector.memset(esel2[0:64, 0:64], 1.0)
        nc.vector.memset(esel2[64:128, 64:128], 1.0)
        # per-head scale row broadcast for the softmax denominators
        selp = [const.tile([128, 128], f32, tag=f"sel{p}", name=f"sel{p}")
                for p in range(2)]
        for p in range(2):
            nc.vector.memset(selp[p], 0.0)
            nc.vector.memset(selp[p][64 * p:64 * p + 1, 0:64], 1.0)
            nc.vector.memset(selp[p][64 * p + 32:64 * p + 33, 64:128], 1.0)
        b_rsq = const.tile([128, 1], f32, tag="brsq")
        nc.vector.memset(b_rsq, 0.5 * math.log(2.0) * (127 + SIGMA + 6))
        b_rcp = const.tile([128, 1], f32, tag="brcp")
        nc.vector.memset(b_rcp, math.log(2.0) * (127 + SIGMA))

        # ---------------- persistent activations / weights / tables
        kk2 = big.tile([128, S], bf16, tag="kk2")
        v_sb = big.tile([128, NJ, 128], bf16, tag="v")
        nc.vector.memset(v_sb, 0.0)
        nc.vector.memset(v_sb[:, :, 64:65], 1.0)
        rkT_sb = big.tile([128, NJ], f32, tag="rkT")
        W_sb = big.tile([128, HC, 512], bf16, tag="W")
        Wo_sb = big.tile([128, 2, H], bf16, tag="Wo")
        cosq_sb = big.tile([128, S], bf16, tag="cosq")
        sinq_sb = big.tile([128, S], bf16, tag="sinq")
        cosk_sb = big.tile([64, S], bf16, tag="cosk")
        sink_sb = big.tile([64, S], bf16, tag="sink")

        xts = {}

        def load_w_x0():
            # critical startup loads: W + x block 0, pair-chunk interleaved;
            # the first pairs ride the HWDGE queues which start ~3us before
            # the gpsimd SWDGE ring comes up.
            xts[0] = []
            for i in range(8):
                xt = xpool.tile([128, 2, 512], bf16, tag="xt", bufs=16,
                                name=f"xt0_{i}")
                if i == 0:
                    # first chunks on the HWDGE queues (flat 2-dim APs),
                    # which start ~3us before the SWDGE ring boots
                    nc.sync.dma_start(out=W_sb[:, 0, :], in_=W_d[0:128, :])
                    nc.scalar.dma_start(out=xt[:, 0, :],
                                        in_=xT_d[0:128, 0:512])
                    nc.sync.dma_start(out=W_sb[:, 1, :], in_=W_d[128:256, :])
                    nc.scalar.dma_start(out=xt[:, 1, :],
                                        in_=xT_d[128:256, 0:512])
                else:
                    nc.gpsimd.dma_start(out=W_sb[:, 2 * i:2 * i + 2, :],
                                        in_=W_r[:, 2 * i:2 * i + 2, :])
                    nc.gpsimd.dma_start(out=xt,
                                        in_=xT_r[:, 2 * i:2 * i + 2, 0:512])
                xts[0].append(xt)

        def load_x(b):
            sp = slice(b * 512, (b + 1) * 512)
            ts = []
            for i in range(8):
                xt = xpool.tile([128, 2, 512], bf16, tag="xt", bufs=16,
                                name=f"xt{b}_{i}")
                nc.gpsimd.dma_start(out=xt, in_=xT_r[:, 2 * i:2 * i + 2, sp])
                ts.append(xt)
            xts[b] = ts

        def load_tables(b):
            # per-block column slices so rope(0) isn't gated on the full set
            sp = slice(b * 512, (b + 1) * 512)

            def pair_src(src_d):
                src = src_d[:, sp]
                return bass.AP(tensor=src.tensor, offset=src.offset,
                               ap=[[0, 2]] + list(src.ap))

            nc.gpsimd.dma_start(out=cosq_sb[:, sp], in_=pair_src(cosq_d))
            nc.gpsimd.dma_start(out=sinq_sb[:, sp], in_=pair_src(sinq_d))
            nc.gpsimd.dma_start(out=cosk_sb[:, sp], in_=cosk_d[:, sp])
            nc.gpsimd.dma_start(out=sink_sb[:, sp], in_=sink_d[:, sp])

        # ---------------- per-block state handed across pipeline stages
        st = {}
        qkv_ps = {}

        def qkv_seg(b, cc, seg):
            """One 4-hc segment of the cc accumulation chain (PE only)."""
            def go():
                if seg == 0:
                    qkv_ps[(b, cc)] = psum.tile([128, 512], f32, tag="qk",
                                                bufs=1, name=f"qk{b}_{cc}")
                ps = qkv_ps[(b, cc)]
                for hc in range(4 * seg, 4 * seg + 4):
                    xt = xts[b][hc // 2][:, hc % 2, :]
                    nc.tensor.matmul(ps[:], W_sb[:, hc, cc * 128:(cc + 1) * 128],
                                     xt, start=(hc == 0), stop=(hc == HC - 1))
            return go

        def extract_q(b, p):
            def go():
                ps = qkv_ps.pop((b, p))
                qr = wkp.tile([128, 512], bf16, tag=f"qr{p}", bufs=2,
                              name=f"qr{b}_{p}")
                nc.vector.tensor_copy(qr, ps[:])
                sq = wkp.tile([128, 512], bf16, tag=f"sq{p}", bufs=2,
                              name=f"sq{b}_{p}")
                nc.vector.tensor_mul(sq, qr, qr)
                st[(b, 'qr', p)] = qr
                st[(b, 'sq', p)] = sq
            return go

        def extract_kv(b):
            def go():
                ps = qkv_ps.pop((b, 2))
                kr = wkp.tile([64, 512], bf16, tag="kr", bufs=2, name=f"kr{b}")
                vr = wkp.tile([64, 512], bf16, tag="vr", bufs=2, name=f"vr{b}")
                nc.scalar.copy(kr, ps[0:64, :])
                nc.scalar.copy(vr, ps[64:128, :])
                ksq = wkp.tile([64, 512], bf16, tag="ksq", bufs=2,
                               name=f"ksq{b}")
                nc.vector.tensor_mul(ksq, kr, kr)
                st[(b, 'kr')] = kr
                st[(b, 'vr')] = vr
                st[(b, 'ksq')] = ksq
            return go

        def extract_gate(b):
            def go():
                ps = qkv_ps.pop((b, 3))
                # exp(-gate): one batched exp over all 128 partitions; the
                # unused rows hold exact zeros (zero gpad cols) -> exp(0)=1
                sig = wkp.tile([128, 512], f32, tag="sig", bufs=2,
                               name=f"sig{b}")
                nc.scalar.activation(sig, ps[:], AF.Exp, scale=-1.0)
                st[(b, 'sig')] = sig
            return go

        def rms_mm(b):
            def go():
                pss = psum.tile([128, 2, 512], f32, tag="sc2", bufs=2,
                                name=f"pss{b}")
                for p in range(2):
                    nc.tensor.matmul(pss[:, p, :], esel2, st.pop((b, 'sq', p)),
                                     start=True, stop=True)
                y0 = wkp.tile([128, 2, 512], f32, tag="y0", bufs=2,
                              name=f"y0{b}")
                nc.scalar.activation(y0, pss[:].bitcast(u32), AF.Exp,
                                     bias=b_rsq, scale=-0.5 * EXPBIT_SCALE)
                st[(b, 'pss')] = pss
                st[(b, 'y0')] = y0
            return go

        def rms_newton(b):
            """One Newton iteration for rq; p=0 on DVE (direct PSUM read),
            p=1 on gpsimd from a staged SBUF copy — the two halves run in
            parallel so the serial rms->rope chain shortens."""
            def go():
                pss = st.pop((b, 'pss'))
                y0 = st.pop((b, 'y0'))
                z0 = wkp.tile([128, 512], f32, tag="z0", bufs=2,
                              name=f"z0_{b}")
                nc.vector.tensor_mul(z0, pss[:, 0, :], y0[:, 0, :])
                nc.vector.tensor_mul(z0, z0, y0[:, 0, :])
                nc.vector.tensor_scalar(z0, z0, -0.5 / HD, 1.5, MUL, ADD)
                nc.vector.tensor_mul(z0, z0, y0[:, 0, :])
                rqs = wkp.tile([128, 512], f32, tag="rqs", bufs=2,
                               name=f"rqs{b}")
                nc.scalar.copy(rqs, pss[:, 1, :])
                z1 = wkp.tile([128, 512], f32, tag="z1", bufs=2,
                              name=f"z1_{b}")
                nc.gpsimd.tensor_mul(z1, rqs, y0[:, 1, :])
                nc.gpsimd.tensor_mul(z1, z1, y0[:, 1, :])
                nc.gpsimd.tensor_scalar(z1, z1, -0.5 / HD, 1.5, MUL, ADD)
                nc.gpsimd.tensor_mul(z1, z1, y0[:, 1, :])
                st[(b, 'rq')] = (z0, z1)
            return go

        def rk_unit(b):
            """k-side sum-squares + Newton -> rkT columns (folds SCALE)."""
            def go():
                ksq = st.pop((b, 'ksq'))
                psr = psum.tile([128, 2, 512], f32, tag="sc2", bufs=2,
                                name=f"psrk{b}")
                for j in range(4):
                    nc.tensor.matmul(psr[:, 0, j:j + 1],
                                     ksq[:, j * 128:(j + 1) * 128],
                                     ones[0:64, :], start=True, stop=True)
                yk = wkp.tile([128, 4], f32, tag="yk", bufs=2, name=f"yk{b}")
                nc.scalar.activation(yk, psr[:, 0, 0:4].bitcast(u32), AF.Exp,
                                     bias=b_rsq, scale=-0.5 * EXPBIT_SCALE)
                for it in range(2):
                    last = (it == 1)
                    tk = wkp.tile([128, 4], f32, tag="tk", bufs=2,
                                  name=f"tk{b}_{it}")
                    nc.vector.tensor_mul(tk, psr[:, 0, 0:4], yk)
                    nc.vector.tensor_mul(tk, tk, yk)
                    nc.vector.tensor_scalar(
                        tk, tk,
                        (-0.5 * SCALE / HD) if last else (-0.5 / HD),
                        (1.5 * SCALE) if last else 1.5, MUL, ADD)
                    if last:
                        nc.vector.tensor_mul(rkT_sb[:, b * 4:(b + 1) * 4],
                                             yk, tk)
                    else:
                        ykn = wkp.tile([128, 4], f32, tag="ykn", bufs=2,
                                       name=f"ykn{b}")
                        nc.vector.tensor_mul(ykn, yk, tk)
                        yk = ykn
            return go

        def vtr(b, j):
            def go():
                J = b * 4 + j
                psv = psum.tile([128, 64], bf16, tag="sc2", bufs=2,
                                name=f"psv{b}_{j}")
                nc.tensor.transpose(psv[:], st[(b, 'vr')][:, j * 128:(j + 1) * 128],
                                    id64)
                nc.scalar.copy(v_sb[:, J, 0:64], psv[:])
            return go

        def rope_stage(b):
            def go():
                qss = []
                for p in range(2):
                    qs = wkp.tile([128, 512], bf16, tag=f"qs{p}", bufs=2,
                                  name=f"qs{b}_{p}")
                    qr = st[(b, 'qr', p)]
                    for g in range(2):
                        bb = g * 64
                        nc.vector.tensor_copy(qs[bb:bb + 32, :],
                                              qr[bb + 32:bb + 64, :])
                        nc.vector.tensor_copy(qs[bb + 32:bb + 64, :],
                                              qr[bb:bb + 32, :])
                    qss.append(qs)
                ks = wkp.tile([64, 512], bf16, tag="ks", bufs=2, name=f"ks{b}")
                kr = st[(b, 'kr')]
                nc.vector.tensor_copy(ks[0:32, :], kr[32:64, :])
                nc.vector.tensor_copy(ks[32:64, :], kr[0:32, :])
                st[(b, 'qs')] = qss
                st[(b, 'ks')] = ks
            return go

        def rope_q(b, p):
            def go():
                sp = slice(b * 512, (b + 1) * 512)
                qr = st.pop((b, 'qr', p))
                eng = nc.vector if p == 0 else nc.gpsimd
                t1 = wkp.tile([128, 512], bf16, tag=f"t1_{p}", bufs=2)
                eng.tensor_mul(t1, qr, cosq_sb[:, sp])
                t2 = wkp.tile([128, 512], bf16, tag=f"t2_{p}", bufs=2)
                eng.tensor_mul(t2, st[(b, 'qs')][p], sinq_sb[:, sp])
                eng.tensor_add(t2, t1, t2)
                qf = wkp.tile([128, 512], bf16, tag=f"qf{p}", bufs=2,
                              name=f"qf{b}_{p}")
                eng.tensor_mul(qf, t2, st[(b, 'rq')][p])
                # hh=1 rows staged at base partition 0 so both score matmuls
                # run as plain base-0 matmuls
                qlo = wkp.tile([64, 512], bf16, tag=f"qlo{p}", bufs=2,
                               name=f"qlo{b}_{p}")
                nc.vector.tensor_copy(qlo, qf[64:128, :])
                st[(b, 'qf', p)] = qf
                st[(b, 'qlo', p)] = qlo
            return go

        def rope_k(b):
            def go():
                sp = slice(b * 512, (b + 1) * 512)
                kr = st.pop((b, 'kr'))
                t1 = wkp.tile([64, 512], bf16, tag="t1k", bufs=2)
                nc.vector.tensor_mul(t1, kr, cosk_sb[:, sp])
                t2 = wkp.tile([64, 512], bf16, tag="t2k", bufs=2)
                nc.vector.tensor_mul(t2, st.pop((b, 'ks')), sink_sb[:, sp])
                nc.vector.tensor_add(kk2[0:64, sp], t1, t2)
            return go

        def finish_p(b, p):
            """Broadcast the packed 1/u scales to head rows, scale PV out."""
            def go():
                sbc = psum.tile([128, 512], f32, tag="sc2", bufs=2,
                                name=f"sbc{b}_{p}")
                nc.tensor.matmul(sbc[:], selp[p], st[(b, 'sy')],
                                 start=True, stop=True)
                at = wkp.tile([128, 512], bf16, tag=f"at{p}", bufs=2,
                              name=f"at{b}_{p}")
                nc.vector.tensor_mul(at, st.pop((b, 'acp', p)), sbc[:])
                st[(b, 'at', p)] = at
            return go

        def op_unit(b, ss, half, alt=False, dve_drain=False):
            """Outproj for si-chunk ss, output cols half*1024:(half+1)*1024.

            alt: rotate pso through the wk+qk banks (qk is idle when there
            is no concurrent QKV chain).  dve_drain: both PSUM drains on
            DVE (used when ACT is the bottleneck engine of the region).
            """
            def go():
                ls = ss * 128
                at0 = st[(b, 'at', 0)]
                at1 = st[(b, 'at', 1)]
                ot = wkp.tile([128, 1024], f32, tag="ot", bufs=3)
                for k in range(2):
                    qtr = 2 * half + k
                    if alt:
                        tg = "wk" if (2 * half + k) % 2 == 0 else "qk"
                        pso = psum.tile([128, 512], f32, tag=tg, bufs=1,
                                        name="pso")
                    else:
                        pso = psum.tile([128, 512], f32, tag="wk", bufs=1,
                                        name="pso")
                    nc.tensor.matmul(pso[:], at0[:, ls:ls + 128],
                                     Wo_sb[:, 0, qtr * 512:(qtr + 1) * 512],
                                     start=True, stop=False)
                    nc.tensor.matmul(pso[:], at1[:, ls:ls + 128],
                                     Wo_sb[:, 1, qtr * 512:(qtr + 1) * 512],
                                     start=False, stop=True)
                    if k == 0 and not dve_drain:
                        nc.scalar.copy(ot[:, 0:512], pso[:])
                    else:
                        nc.vector.tensor_copy(ot[:, k * 512:(k + 1) * 512],
                                              pso[:])
                r0 = (4 * b + ss) * 128
                nc.gpsimd.dma_start(
                    out=out_d[r0:r0 + 128, half * 1024:(half + 1) * 1024],
                    in_=ot)
            return go

        def attention(B, fill):
            """J-loop spine; pops filler closures between iterations."""
            u_q = wkp.tile([128, 512], f32, tag="u", bufs=2, name=f"u{B}")
            nc.vector.memset(u_q, 1.0)
            total = len(fill)
            iters = 2 * (4 * B + 4)
            done = 0
            emitted = 0
            for p in range(2):
                ps_att = psum.tile([128, 2, 512], f32, tag="att", bufs=1,
                                   name=f"psatt{B}_{p}")
                qf = st[(B, 'qf', p)]
                qlo = st[(B, 'qlo', p)]
                for J in range(4 * B + 4):
                    off = max(0, (J - 4 * B) * 128)
                    pss = psum.tile([128, 2, 512], f32, tag="sc2", bufs=2,
                                    name="pscr")
                    kst = kk2[0:64, J * 128:(J + 1) * 128]
                    nc.tensor.matmul(pss[:, 0, off:512], kst,
                                     qf[0:64, off:512], start=True, stop=True)
                    nc.tensor.matmul(pss[:, 1, off:512], kst,
                                     qlo[:, off:512], start=True, stop=True)
                    et = wkp.tile([128, 2, 512], bf16, tag="et", bufs=6,
                                  name="et")
                    nc.scalar.activation(et[:, :, off:512], pss[:, :, off:512],
                                         AF.Exp, scale=rkT_sb[:, J:J + 1])
                    if off > 0 or J == 4 * B:
                        nc.vector.tensor_mul(et[:, 0, off:off + 128],
                                             et[:, 0, off:off + 128], tri)
                        nc.vector.tensor_mul(et[:, 1, off:off + 128],
                                             et[:, 1, off:off + 128], tri)
                    for hh in range(2):
                        nc.tensor.matmul(ps_att[:, hh, off:512], v_sb[:, J, :],
                                         et[:, hh, off:512],
                                         start=(J == 0), stop=(J == 4 * B + 3))
                    done += 1
                    want = (total * done + iters - 1) // iters
                    while emitted < want and fill:
                        fill.pop(0)()
                        emitted += 1
                # drain PV: values to SBUF (ACT) + u=(1+exp(-g))*den (DVE)
                acp = wkp.tile([128, 512], f32, tag="acp", bufs=3,
                               name=f"acp{B}_{p}")
                if B == NB - 1:
                    nc.vector.tensor_copy(acp[0:64, :], ps_att[0:64, 0, :])
                    nc.vector.tensor_copy(acp[64:128, :], ps_att[0:64, 1, :])
                else:
                    nc.scalar.copy(acp[0:64, :], ps_att[0:64, 0, :])
                    nc.scalar.copy(acp[64:128, :], ps_att[0:64, 1, :])
                st[(B, 'acp', p)] = acp
                sig = st[(B, 'sig')]
                for hh in range(2):
                    r = 64 * p + 32 * hh
                    nc.vector.scalar_tensor_tensor(u_q[r:r + 1, :],
                                                   sig[r:r + 1, :], 1.0,
                                                   ps_att[64:65, hh, :],
                                                   ADD, MUL)
            # packed Newton reciprocal for all four denominators
            s_y = wkp.tile([128, 512], f32, tag="sy", bufs=2, name=f"sy{B}")
            nc.scalar.activation(s_y, u_q[:].bitcast(u32), AF.Exp,
                                 bias=b_rcp, scale=-EXPBIT_SCALE)
            tu = wkp.tile([128, 512], f32, tag="tu", bufs=2, name=f"tu{B}")
            nc.vector.tensor_mul(tu, u_q, s_y)
            nc.vector.tensor_scalar(tu, tu, -1.0, 2.0, MUL, ADD)
            nc.vector.tensor_mul(s_y, s_y, tu)
            st[(B, 'sy')] = s_y
            while fill:
                fill.pop(0)()

        def qkv_block_fill(b):
            """Filler closures that compute block b's QKV/rms/rope."""
            fl = []
            fl += [qkv_seg(b, 0, s) for s in range(4)] + [extract_q(b, 0)]
            fl += [qkv_seg(b, 1, s) for s in range(4)] + [extract_q(b, 1)]
            fl += [qkv_seg(b, 2, s) for s in range(4)] + [extract_kv(b)]
            fl += [rms_mm(b), rms_newton(b)]
            fl += [qkv_seg(b, 3, s) for s in range(4)] + [extract_gate(b)]
            fl += [rk_unit(b)]
            fl += [vtr(b, j) for j in range(4)]
            fl += [rope_stage(b), rope_q(b, 0), rope_k(b), rope_q(b, 1)]
            return fl

        def interleave(a, bl):
            """Round-robin merge keeping each list's internal order."""
            out = []
            ia = ib = 0
            na, nbl = len(a), len(bl)
            tot = na + nbl
            for k in range(tot):
                # proportional progress
                if ia * nbl <= ib * na and ia < na:
                    out.append(a[ia]); ia += 1
                elif ib < nbl:
                    out.append(bl[ib]); ib += 1
                else:
                    out.append(a[ia]); ia += 1
            return out

        # ---------------- schedule
        load_w_x0()
        make_identity(nc, id64)
        make_upper_triangular(nc, tri, val=1.0, diag=True)
        load_tables(0)
        load_x(1)
        load_tables(1)
        nc.gpsimd.dma_start(out=Wo_sb, in_=Wo_d.ap().rearrange(
            "(cc p) h -> p cc h", p=128))
        # prologue: block 0 QKV, hc-major so the matmuls track the
        # streaming W/x chunk arrivals (4 accumulators in two sc2 tiles)
        psA = psum.tile([128, 2, 512], f32, tag="sc2", bufs=2, name="proA")
        psB = psum.tile([128, 2, 512], f32, tag="sc2", bufs=2, name="proB")
        for hc in range(HC):
            xt = xts[0][hc // 2][:, hc % 2, :]
            fst = (hc == 0)
            fin = (hc == HC - 1)
            nc.tensor.matmul(psA[:, 0, :], W_sb[:, hc, 0:128], xt,
                             start=fst, stop=fin)
            nc.tensor.matmul(psA[:, 1, :], W_sb[:, hc, 128:256], xt,
                             start=fst, stop=fin)
            nc.tensor.matmul(psB[:, 0, :], W_sb[:, hc, 256:384], xt,
                             start=fst, stop=fin)
            nc.tensor.matmul(psB[:, 1, :], W_sb[:, hc, 384:512], xt,
                             start=fst, stop=fin)
        for p in range(2):
            qr = wkp.tile([128, 512], bf16, tag=f"qr{p}", bufs=2,
                          name=f"qr0_{p}")
            nc.vector.tensor_copy(qr, psA[:, p, :])
            sq = wkp.tile([128, 512], bf16, tag=f"sq{p}", bufs=2,
                          name=f"sq0_{p}")
            nc.vector.tensor_mul(sq, qr, qr)
            st[(0, 'qr', p)] = qr
            st[(0, 'sq', p)] = sq
        kr = wkp.tile([64, 512], bf16, tag="kr", bufs=2, name="kr0")
        vr = wkp.tile([64, 512], bf16, tag="vr", bufs=2, name="vr0")
        nc.scalar.copy(kr, psB[0:64, 0, :])
        nc.scalar.copy(vr, psB[64:128, 0, :])
        ksq = wkp.tile([64, 512], bf16, tag="ksq", bufs=2, name="ksq0")
        nc.vector.tensor_mul(ksq, kr, kr)
        st[(0, 'kr')] = kr
        st[(0, 'vr')] = vr
        st[(0, 'ksq')] = ksq
        sig = wkp.tile([128, 512], f32, tag="sig", bufs=2, name="sig0")
        nc.scalar.activation(sig, psB[:, 1, :], AF.Exp, scale=-1.0)
        st[(0, 'sig')] = sig
        rms_mm(0)()
        rms_newton(0)()
        rk_unit(0)()
        for j in range(4):
            vtr(0, j)()
        rope_stage(0)()
        rope_q(0, 0)()
        rope_k(0)()
        rope_q(0, 1)()
        for B in range(NB):
            if B + 2 < NB:
                load_x(B + 2)
                load_tables(B + 2)
            last = (B == NB - 1)
            fill = []
            if B >= 1:
                fill += [finish_p(B - 1, 0), finish_p(B - 1, 1)]
            qf_fill = qkv_block_fill(B + 1) if B + 1 < NB else []
            op_fill = ([op_unit(B - 1, ss, half, alt=last, dve_drain=last)
                        for ss in range(4) for half in range(2)]
                       if B >= 1 else [])
            fill += interleave(qf_fill, op_fill)
            attention(B, fill)
        # tail
        finish_p(NB - 1, 0)()
        finish_p(NB - 1, 1)()
        for ss in range(4):
            for half in range(2):
                op_unit(NB - 1, ss, half, alt=True)()

    nc.compile()
    return nc


def _get_nc():
    if "nc" not in _BUILT:
        _BUILT["nc"] = _build_nc()
    return _BUILT["nc"]


# ---------------------------------------------------------------- entry point
def _install_ntff_hook():
    import types
    try:
        import antenv
        if "antenv.axon_hooks" in sys.modules:
            return True
        mod = types.ModuleType("antenv.axon_hooks")
        holder = [None]
        mod.set_axon_ntff_profile_hook = lambda h: holder.__setitem__(0, h)
        mod.get_axon_ntff_profile_hook = lambda: holder[0]
        sys.modules["antenv.axon_hooks"] = mod
        antenv.axon_hooks = mod
        from trn_agent_boot.trn_boot import _ntff_profile_via_ctypes
        hook = _ntff_profile_via_ctypes("/opt/axon/libaxon_pjrt.so")
        if hook is None:
            return False
        mod.set_axon_ntff_profile_hook(hook)
        return True
    except Exception:
        return False


def kernel(hidden_states, Wq, Wk, Wv, Wo, g_q, g_k):
    global LAST_EXEC_NS
    from concourse.bass_utils import run_bass_kernel_spmd

    in_maps = _host_prep(hidden_states, Wq, Wk, Wv, Wo, g_q, g_k)
    nc = _get_nc()
    trace = os.environ.get("KERNEL_TRACE", "0") == "1"
    if trace:
        trace = _install_ntff_hook()
    res = run_bass_kernel_spmd(nc, in_maps, list(range(NCORES)), trace=trace)
    LAST_EXEC_NS = res.exec_time_ns
    out = np.zeros((S, H), np.float32)
    for c in range(NCORES):
        out += res.results[c]["out"]
    return out.reshape(1, S, H).astype(np.float32)


# revision 11
# speedup vs baseline: 1.2926x; 1.2926x over previous
"""GatedAttention TRN2 kernel — 8-core tensor-parallel, fused-pipeline v2.

Self-contained: host-side shard/layout prep + Bass/Tile kernel + gather.
One kv-head group (4 q heads) per core; host sums the 8 output partials.

v2 restructure vs v1: the per-block phases (QKV projection, attention,
output projection) no longer run as separate engine-serial stretches.
Instead, attention(B)'s J-loop is the spine, and QKV(B+1) / outproj(B-1)
matmuls are emitted as fine-grained filler between J iterations, so the
PE streams continuously while ACT drains softmax exps:
  - QKV runs as 4 per-cc accumulation chains (16 hc MMs each) into a
    single rotating PSUM bank; each chain is extracted immediately
    (q -> DVE, k/v -> ACT, gate -> one batched ACT exp over the whole
    [128,512] tile instead of 4 single-row exps).
  - softmax exps are hh-batched: both heads of a (p,J) pair land in one
    [128,2,512] two-bank PSUM tile and one ACT exp (FD up to 1024) with
    the shared per-partition k-rms scale.
  - q-side RMS Newton runs once per block on [128,1024] tiles (both p
    halves) with in-place DVE chains.
  - PSUM budget (8 banks): scores/transients [128,1024]x2 (4) +
    ps_att [128,2,512] (2) + qkv chain (1) + outproj (1).
  - bulk DMA descriptors batched ([128,2,512] x-pairs, [128,1024] out
    stores) to cut gpsimd SWDGE descriptor time; first W/x chunks ride
    the 2 HWDGE queues (sync/scalar) which start ~3us earlier.
Engine-SBUF access patterns must start at partitions 0/32/64/96; PSUM
row slices are exempt. Cross-partition copies are DVE-only.
"""
import math
import os
import sys
import numpy as np
import ml_dtypes

BF16 = ml_dtypes.bfloat16

H, NH, KVH, HD = 2048, 32, 8, 64
G = NH // KVH          # 4 q heads per core
S = 2048
EPS = 1e-6
THETA = 1000000.0
SCALE = 1.0 / math.sqrt(HD)
NCORES = 8
HC = H // 128          # 16 h-chunks
NB = S // 512          # 4 si-blocks
NJ = S // 128          # 16 sj-chunks

_BUILT = {}
LAST_EXEC_NS = None


# ---------------------------------------------------------------- host prep
def _host_prep(hidden_states, Wq, Wk, Wv, Wo, g_q, g_k):
    x = np.ascontiguousarray(np.asarray(hidden_states, np.float32).reshape(S, H))
    Wq = np.asarray(Wq, np.float32)
    Wk = np.asarray(Wk, np.float32)
    Wv = np.asarray(Wv, np.float32)
    Wo = np.asarray(Wo, np.float32)
    g_q = np.asarray(g_q, np.float32)
    g_k = np.asarray(g_k, np.float32)

    xT = np.ascontiguousarray(x.T).astype(BF16)

    inv_freq = 1.0 / (THETA ** (np.arange(0, HD, 2, dtype=np.float32) / HD))
    pos = np.arange(S, dtype=np.float32)
    emb = np.concatenate([pos[:, None] * inv_freq[None, :]] * 2, axis=-1)  # [S,64]
    cos = np.cos(emb).T.astype(np.float32)   # [64, S]
    sin = np.sin(emb).T.astype(np.float32)
    sign = np.where(np.arange(HD) < HD // 2, -1.0, 1.0).astype(np.float32)[:, None]
    cosq = np.ascontiguousarray(cos * g_q[:, None]).astype(BF16)
    sinq = np.ascontiguousarray(sin * sign * np.roll(g_q, -32)[:, None]).astype(BF16)
    cosk = np.ascontiguousarray(cos * g_k[:, None]).astype(BF16)
    sink = np.ascontiguousarray(sin * sign * np.roll(g_k, -32)[:, None]).astype(BF16)

    in_maps = []
    for c in range(NCORES):
        Wq_g = Wq[:, c * (G * HD + G):(c + 1) * (G * HD + G)]
        gpad = np.zeros((H, 128), np.float32)
        for p in range(2):
            for hh in range(2):
                # gate for head (p,hh) lands on PSUM partition 64p+32hh — a
                # legal SBUF partition start for the sig_q reads downstream
                gpad[:, 64 * p + 32 * hh] = Wq_g[:, G * HD + 2 * p + hh]
        W_c = np.ascontiguousarray(np.concatenate(
            [Wq_g[:, :G * HD],
             Wk[:, c * HD:(c + 1) * HD],
             Wv[:, c * HD:(c + 1) * HD],
             gpad], axis=1))                                   # [H, 512]
        Wo_c = np.ascontiguousarray(Wo[c * G * HD:(c + 1) * G * HD, :])  # [256,H]
        in_maps.append({"xT": xT, "W": W_c.astype(BF16), "Wo": Wo_c.astype(BF16),
                        "cosq": cosq, "sinq": sinq, "cosk": cosk, "sink": sink})
    return in_maps


# ---------------------------------------------------------------- bass build
def _build_nc():
    import concourse.bass as bass
    import concourse.mybir as mybir
    import concourse.tile as tile
    from concourse import bacc
    from concourse.masks import make_identity, make_upper_triangular

    dt = mybir.dt
    f32 = dt.float32
    bf16 = dt.bfloat16
    u32 = dt.uint32
    AF = mybir.ActivationFunctionType
    MUL = mybir.AluOpType.mult
    ADD = mybir.AluOpType.add

    nc = bacc.Bacc("TRN2", target_bir_lowering=False, debug=False,
                   num_devices=NCORES)

    xT_d = nc.dram_tensor("xT", [H, S], bf16, kind="ExternalInput")
    W_d = nc.dram_tensor("W", [H, 512], bf16, kind="ExternalInput")
    Wo_d = nc.dram_tensor("Wo", [G * HD, H], bf16, kind="ExternalInput")
    cosq_d = nc.dram_tensor("cosq", [HD, S], bf16, kind="ExternalInput")
    sinq_d = nc.dram_tensor("sinq", [HD, S], bf16, kind="ExternalInput")
    cosk_d = nc.dram_tensor("cosk", [HD, S], bf16, kind="ExternalInput")
    sink_d = nc.dram_tensor("sink", [HD, S], bf16, kind="ExternalInput")
    out_d = nc.dram_tensor("out", [S, H], bf16, kind="ExternalOutput")

    xT_r = xT_d.ap().rearrange("(c p) s -> p c s", p=128)    # [128,16,S]
    W_r = W_d.ap().rearrange("(c p) w -> p c w", p=128)      # [128,16,512]

    SIGMA = 0.0430
    EXPBIT_SCALE = math.log(2.0) / (1 << 23)

    import contextlib
    with tile.TileContext(nc) as tc, contextlib.ExitStack() as ctx:
        const = ctx.enter_context(tc.tile_pool(name="const", bufs=1))
        big = ctx.enter_context(tc.tile_pool(name="big", bufs=1))
        xpool = ctx.enter_context(tc.tile_pool(name="xp", bufs=16))
        wkp = ctx.enter_context(tc.tile_pool(name="wkp", bufs=2))
        psum = ctx.enter_context(tc.tile_pool(name="ps", bufs=1, space="PSUM"))

        # ---------------- constants
        id64 = const.tile([64, 64], bf16, tag="id64")
        tri = const.tile([128, 128], bf16, tag="tri")
        ones = const.tile([128, 1], bf16, tag="ones")
        nc.vector.memset(ones, 1.0)
        # block-diagonal selector: sums 64-row head blocks AND broadcasts the
        # sum back to all 64 rows of the head
        esel2 = const.tile([128, 128], bf16, tag="esel2")
        nc.vector.memset(esel2, 0.0)
        nc.vector.memset(esel2[0:64, 0:64], 1.0)
        nc.vector.memset(esel2[64:128, 64:128], 1.0)
        # per-head scale row broadcast for the softmax denominators
        selp = [const.tile([128, 128], f32, tag=f"sel{p}", name=f"sel{p}")
                for p in range(2)]
        for p in range(2):
            nc.vector.memset(selp[p], 0.0)
            nc.vector.memset(selp[p][64 * p:64 * p + 1, 0:64], 1.0)
            nc.vector.memset(selp[p][64 * p + 32:64 * p + 33, 64:128], 1.0)
        b_rsq = const.tile([128, 1], f32, tag="brsq")
        nc.vector.memset(b_rsq, 0.5 * math.log(2.0) * (127 + SIGMA + 6))
        b_rcp = const.tile([128, 1], f32, tag="brcp")
        nc.vector.memset(b_rcp, math.log(2.0) * (127 + SIGMA))

        # ---------------- persistent activations / weights / tables
        kk2 = big.tile([128, S], bf16, tag="kk2")
        v_sb = big.tile([128, NJ, 128], bf16, tag="v")
        nc.vector.memset(v_sb, 0.0)
        nc.vector.memset(v_sb[:, :, 64:65], 1.0)
        rkT_sb = big.tile([128, NJ], f32, tag="rkT")
        W_sb = big.tile([128, HC, 512], bf16, tag="W")
        Wo_sb = big.tile([128, 2, H], bf16, tag="Wo")
        cosq_sb = big.tile([128, S], bf16, tag="cosq")
        sinq_sb = big.tile([128, S], bf16, tag="sinq")
        cosk_sb = big.tile([64, S], bf16, tag="cosk")
        sink_sb = big.tile([64, S], bf16, tag="sink")

        xts = {}

        def load_w_x0():
            # critical startup loads: W + x block 0, pair-chunk interleaved;
            # the first pairs ride the HWDGE queues which start ~3us before
            # the gpsimd SWDGE ring comes up.
            xts[0] = []
            for i in range(8):
                xt = xpool.tile([128, 2, 512], bf16, tag="xt", bufs=16,
                                name=f"xt0_{i}")
                if i == 0:
                    # first chunks on the HWDGE queues (flat 2-dim APs),
                    # which start ~3us before the SWDGE ring boots
                    nc.sync.dma_start(out=W_sb[:, 0, :], in_=W_d[0:128, :])
                    nc.scalar.dma_start(out=xt[:, 0, :],
                                        in_=xT_d[0:128, 0:512])
                    nc.sync.dma_start(out=W_sb[:, 1, :], in_=W_d[128:256, :])
                    nc.scalar.dma_start(out=xt[:, 1, :],
                                        in_=xT_d[128:256, 0:512])
                else:
                    nc.gpsimd.dma_start(out=W_sb[:, 2 * i:2 * i + 2, :],
                                        in_=W_r[:, 2 * i:2 * i + 2, :])
                    nc.gpsimd.dma_start(out=xt,
                                        in_=xT_r[:, 2 * i:2 * i + 2, 0:512])
                xts[0].append(xt)

        def load_x(b):
            sp = slice(b * 512, (b + 1) * 512)
            ts = []
            for i in range(8):
                xt = xpool.tile([128, 2, 512], bf16, tag="xt", bufs=16,
                                name=f"xt{b}_{i}")
                nc.gpsimd.dma_start(out=xt, in_=xT_r[:, 2 * i:2 * i + 2, sp])
                ts.append(xt)
            xts[b] = ts

        def load_tables(b):
            # per-block column slices so rope(0) isn't gated on the full set
            sp = slice(b * 512, (b + 1) * 512)

            def pair_src(src_d):
                src = src_d[:, sp]
                return bass.AP(tensor=src.tensor, offset=src.offset,
                               ap=[[0, 2]] + list(src.ap))

            nc.gpsimd.dma_start(out=cosq_sb[:, sp], in_=pair_src(cosq_d))
            nc.gpsimd.dma_start(out=sinq_sb[:, sp], in_=pair_src(sinq_d))
            nc.gpsimd.dma_start(out=cosk_sb[:, sp], in_=cosk_d[:, sp])
            nc.gpsimd.dma_start(out=sink_sb[:, sp], in_=sink_d[:, sp])

        # ---------------- per-block state handed across pipeline stages
        st = {}
        qkv_ps = {}

        def qkv_seg(b, cc, seg):
            """One 4-hc segment of the cc accumulation chain (PE only)."""
            def go():
                if seg == 0:
                    qkv_ps[(b, cc)] = psum.tile([128, 512], f32, tag="qk",
                                                bufs=1, name=f"qk{b}_{cc}")
                ps = qkv_ps[(b, cc)]
                for hc in range(4 * seg, 4 * seg + 4):
                    xt = xts[b][hc // 2][:, hc % 2, :]
                    nc.tensor.matmul(ps[:], W_sb[:, hc, cc * 128:(cc + 1) * 128],
                                     xt, start=(hc == 0), stop=(hc == HC - 1))
            return go

        def extract_q(b, p):
            def go():
                ps = qkv_ps.pop((b, p))
                qr = wkp.tile([128, 512], bf16, tag=f"qr{p}", bufs=2,
                              name=f"qr{b}_{p}")
                nc.vector.tensor_copy(qr, ps[:])
                sq = wkp.tile([128, 512], bf16, tag=f"sq{p}", bufs=2,
                              name=f"sq{b}_{p}")
                nc.vector.tensor_mul(sq, qr, qr)
                st[(b, 'qr', p)] = qr
                st[(b, 'sq', p)] = sq
            return go

        def extract_kv(b):
            def go():
                ps = qkv_ps.pop((b, 2))
                kr = wkp.tile([64, 512], bf16, tag="kr", bufs=2, name=f"kr{b}")
                vr = wkp.tile([64, 512], bf16, tag="vr", bufs=2, name=f"vr{b}")
                nc.scalar.copy(kr, ps[0:64, :])
                nc.scalar.copy(vr, ps[64:128, :])
                ksq = wkp.tile([64, 512], bf16, tag="ksq", bufs=2,
                               name=f"ksq{b}")
                nc.vector.tensor_mul(ksq, kr, kr)
                st[(b, 'kr')] = kr
                st[(b, 'vr')] = vr
                st[(b, 'ksq')] = ksq
            return go

        def extract_gate(b):
            def go():
                ps = qkv_ps.pop((b, 3))
                # exp(-gate): one batched exp over all 128 partitions; the
                # unused rows hold exact zeros (zero gpad cols) -> exp(0)=1
                sig = wkp.tile([128, 512], f32, tag="sig", bufs=2,
                               name=f"sig{b}")
                nc.scalar.activation(sig, ps[:], AF.Exp, scale=-1.0)
                st[(b, 'sig')] = sig
            return go

        def rms_mm(b):
            def go():
                pss = psum.tile([128, 2, 512], f32, tag="sc2", bufs=2,
                                name=f"pss{b}")
                for p in range(2):
                    nc.tensor.matmul(pss[:, p, :], esel2, st.pop((b, 'sq', p)),
                                     start=True, stop=True)
                y0 = wkp.tile([128, 2, 512], f32, tag="y0", bufs=2,
                              name=f"y0{b}")
                nc.scalar.activation(y0, pss[:].bitcast(u32), AF.Exp,
                                     bias=b_rsq, scale=-0.5 * EXPBIT_SCALE)
                st[(b, 'pss')] = pss
                st[(b, 'y0')] = y0
            return go

        def rms_newton(b):
            """One Newton iteration for rq on both p halves at once (DVE)."""
            def go():
                pss = st.pop((b, 'pss'))
                y0 = st.pop((b, 'y0'))
                z = wkp.tile([128, 2, 512], f32, tag="z", bufs=2, name=f"z{b}")
                nc.vector.tensor_mul(z, pss[:], y0)       # frees the banks
                nc.vector.tensor_mul(z, z, y0)            # a*y0^2
                nc.vector.tensor_scalar(z, z, -0.5 / HD, 1.5, MUL, ADD)
                nc.vector.tensor_mul(z, z, y0)            # rq = y0*(1.5-...)
                st[(b, 'rq')] = z
            return go

        def rk_unit(b):
            """k-side sum-squares + Newton -> rkT columns (folds SCALE)."""
            def go():
                ksq = st.pop((b, 'ksq'))
                psr = psum.tile([128, 2, 512], f32, tag="sc2", bufs=2,
                                name=f"psrk{b}")
                for j in range(4):
                    nc.tensor.matmul(psr[:, 0, j:j + 1],
                                     ksq[:, j * 128:(j + 1) * 128],
                                     ones[0:64, :], start=True, stop=True)
                yk = wkp.tile([128, 4], f32, tag="yk", bufs=2, name=f"yk{b}")
                nc.scalar.activation(yk, psr[:, 0, 0:4].bitcast(u32), AF.Exp,
                                     bias=b_rsq, scale=-0.5 * EXPBIT_SCALE)
                for it in range(2):
                    last = (it == 1)
                    tk = wkp.tile([128, 4], f32, tag="tk", bufs=2,
                                  name=f"tk{b}_{it}")
                    nc.vector.tensor_mul(tk, psr[:, 0, 0:4], yk)
                    nc.vector.tensor_mul(tk, tk, yk)
                    nc.vector.tensor_scalar(
                        tk, tk,
                        (-0.5 * SCALE / HD) if last else (-0.5 / HD),
                        (1.5 * SCALE) if last else 1.5, MUL, ADD)
                    if last:
                        nc.vector.tensor_mul(rkT_sb[:, b * 4:(b + 1) * 4],
                                             yk, tk)
                    else:
                        ykn = wkp.tile([128, 4], f32, tag="ykn", bufs=2,
                                       name=f"ykn{b}")
                        nc.vector.tensor_mul(ykn, yk, tk)
                        yk = ykn
            return go

        def vtr(b, j):
            def go():
                J = b * 4 + j
                psv = psum.tile([128, 64], bf16, tag="sc2", bufs=2,
                                name=f"psv{b}_{j}")
                nc.tensor.transpose(psv[:], st[(b, 'vr')][:, j * 128:(j + 1) * 128],
                                    id64)
                nc.scalar.copy(v_sb[:, J, 0:64], psv[:])
            return go

        def rope_stage(b):
            def go():
                qss = []
                for p in range(2):
                    qs = wkp.tile([128, 512], bf16, tag=f"qs{p}", bufs=2,
                                  name=f"qs{b}_{p}")
                    qr = st[(b, 'qr', p)]
                    for g in range(2):
                        bb = g * 64
                        nc.vector.tensor_copy(qs[bb:bb + 32, :],
                                              qr[bb + 32:bb + 64, :])
                        nc.vector.tensor_copy(qs[bb + 32:bb + 64, :],
                                              qr[bb:bb + 32, :])
                    qss.append(qs)
                ks = wkp.tile([64, 512], bf16, tag="ks", bufs=2, name=f"ks{b}")
                kr = st[(b, 'kr')]
                nc.vector.tensor_copy(ks[0:32, :], kr[32:64, :])
                nc.vector.tensor_copy(ks[32:64, :], kr[0:32, :])
                st[(b, 'qs')] = qss
                st[(b, 'ks')] = ks
            return go

        def rope_q(b, p):
            def go():
                sp = slice(b * 512, (b + 1) * 512)
                qr = st.pop((b, 'qr', p))
                t1 = wkp.tile([128, 512], bf16, tag="t1", bufs=2)
                nc.vector.tensor_mul(t1, qr, cosq_sb[:, sp])
                t2 = wkp.tile([128, 512], bf16, tag="t2", bufs=2)
                nc.vector.tensor_mul(t2, st[(b, 'qs')][p], sinq_sb[:, sp])
                nc.vector.tensor_add(t2, t1, t2)
                qf = wkp.tile([128, 512], bf16, tag=f"qf{p}", bufs=2,
                              name=f"qf{b}_{p}")
                nc.vector.tensor_mul(qf, t2, st[(b, 'rq')][:, p, :])
                # hh=1 rows staged at base partition 0 so both score matmuls
                # run as plain base-0 matmuls
                qlo = wkp.tile([64, 512], bf16, tag=f"qlo{p}", bufs=2,
                               name=f"qlo{b}_{p}")
                nc.vector.tensor_copy(qlo, qf[64:128, :])
                st[(b, 'qf', p)] = qf
                st[(b, 'qlo', p)] = qlo
            return go

        def rope_k(b):
            def go():
                sp = slice(b * 512, (b + 1) * 512)
                kr = st.pop((b, 'kr'))
                t1 = wkp.tile([64, 512], bf16, tag="t1k", bufs=2)
                nc.vector.tensor_mul(t1, kr, cosk_sb[:, sp])
                t2 = wkp.tile([64, 512], bf16, tag="t2k", bufs=2)
                nc.vector.tensor_mul(t2, st.pop((b, 'ks')), sink_sb[:, sp])
                nc.vector.tensor_add(kk2[0:64, sp], t1, t2)
            return go

        def finish_p(b, p):
            """Broadcast the packed 1/u scales to head rows, scale PV out."""
            def go():
                sbc = psum.tile([128, 512], f32, tag="sc2", bufs=2,
                                name=f"sbc{b}_{p}")
                nc.tensor.matmul(sbc[:], selp[p], st[(b, 'sy')],
                                 start=True, stop=True)
                at = wkp.tile([128, 512], bf16, tag=f"at{p}", bufs=2,
                              name=f"at{b}_{p}")
                nc.vector.tensor_mul(at, st.pop((b, 'acp', p)), sbc[:])
                st[(b, 'at', p)] = at
            return go

        def op_unit(b, ss, half, alt=False, dve_drain=False):
            """Outproj for si-chunk ss, output cols half*1024:(half+1)*1024.

            alt: rotate pso through the wk+qk banks (qk is idle when there
            is no concurrent QKV chain).  dve_drain: both PSUM drains on
            DVE (used when ACT is the bottleneck engine of the region).
            """
            def go():
                ls = ss * 128
                at0 = st[(b, 'at', 0)]
                at1 = st[(b, 'at', 1)]
                ot = wkp.tile([128, 1024], bf16, tag="ot", bufs=3)
                for k in range(2):
                    qtr = 2 * half + k
                    if alt:
                        tg = "wk" if (2 * half + k) % 2 == 0 else "qk"
                        pso = psum.tile([128, 512], f32, tag=tg, bufs=1,
                                        name="pso")
                    else:
                        pso = psum.tile([128, 512], f32, tag="wk", bufs=1,
                                        name="pso")
                    nc.tensor.matmul(pso[:], at0[:, ls:ls + 128],
                                     Wo_sb[:, 0, qtr * 512:(qtr + 1) * 512],
                                     start=True, stop=False)
                    nc.tensor.matmul(pso[:], at1[:, ls:ls + 128],
                                     Wo_sb[:, 1, qtr * 512:(qtr + 1) * 512],
                                     start=False, stop=True)
                    if k == 0 and not dve_drain:
                        nc.scalar.copy(ot[:, 0:512], pso[:])
                    else:
                        nc.vector.tensor_copy(ot[:, k * 512:(k + 1) * 512],
                                              pso[:])
                r0 = (4 * b + ss) * 128
                nc.gpsimd.dma_start(
                    out=out_d[r0:r0 + 128, half * 1024:(half + 1) * 1024],
                    in_=ot)
            return go

        def attention(B, fill):
            """J-loop spine; pops filler closures between iterations."""
            u_q = wkp.tile([128, 512], f32, tag="u", bufs=2, name=f"u{B}")
            nc.vector.memset(u_q, 1.0)
            total = len(fill)
            iters = 2 * (4 * B + 4)
            done = 0
            emitted = 0
            for p in range(2):
                ps_att = psum.tile([128, 2, 512], f32, tag="att", bufs=1,
                                   name=f"psatt{B}_{p}")
                qf = st[(B, 'qf', p)]
                qlo = st[(B, 'qlo', p)]
                for J in range(4 * B + 4):
                    off = max(0, (J - 4 * B) * 128)
                    pss = psum.tile([128, 2, 512], f32, tag="sc2", bufs=2,
                                    name="pscr")
                    kst = kk2[0:64, J * 128:(J + 1) * 128]
                    nc.tensor.matmul(pss[:, 0, off:512], kst,
                                     qf[0:64, off:512], start=True, stop=True)
                    nc.tensor.matmul(pss[:, 1, off:512], kst,
                                     qlo[:, off:512], start=True, stop=True)
                    et = wkp.tile([128, 2, 512], bf16, tag="et", bufs=6,
                                  name="et")
                    nc.scalar.activation(et[:, :, off:512], pss[:, :, off:512],
                                         AF.Exp, scale=rkT_sb[:, J:J + 1])
                    if off > 0 or J == 4 * B:
                        nc.vector.tensor_mul(et[:, 0, off:off + 128],
                                             et[:, 0, off:off + 128], tri)
                        nc.vector.tensor_mul(et[:, 1, off:off + 128],
                                             et[:, 1, off:off + 128], tri)
                    for hh in range(2):
                        nc.tensor.matmul(ps_att[:, hh, off:512], v_sb[:, J, :],
                                         et[:, hh, off:512],
                                         start=(J == 0), stop=(J == 4 * B + 3))
                    done += 1
                    want = (total * done + iters - 1) // iters
                    while emitted < want and fill:
                        fill.pop(0)()
                        emitted += 1
                # drain PV: values to SBUF (ACT) + u=(1+exp(-g))*den (DVE)
                acp = wkp.tile([128, 512], f32, tag="acp", bufs=3,
                               name=f"acp{B}_{p}")
                if B == NB - 1:
                    nc.vector.tensor_copy(acp[0:64, :], ps_att[0:64, 0, :])
                    nc.vector.tensor_copy(acp[64:128, :], ps_att[0:64, 1, :])
                else:
                    nc.scalar.copy(acp[0:64, :], ps_att[0:64, 0, :])
                    nc.scalar.copy(acp[64:128, :], ps_att[0:64, 1, :])
                st[(B, 'acp', p)] = acp
                sig = st[(B, 'sig')]
                for hh in range(2):
                    r = 64 * p + 32 * hh
                    nc.vector.scalar_tensor_tensor(u_q[r:r + 1, :],
                                                   sig[r:r + 1, :], 1.0,
                                                   ps_att[64:65, hh, :],
                                                   ADD, MUL)
            # packed Newton reciprocal for all four denominators
            s_y = wkp.tile([128, 512], f32, tag="sy", bufs=2, name=f"sy{B}")
            nc.scalar.activation(s_y, u_q[:].bitcast(u32), AF.Exp,
                                 bias=b_rcp, scale=-EXPBIT_SCALE)
            tu = wkp.tile([128, 512], f32, tag="tu", bufs=2, name=f"tu{B}")
            nc.vector.tensor_mul(tu, u_q, s_y)
            nc.vector.tensor_scalar(tu, tu, -1.0, 2.0, MUL, ADD)
            nc.vector.tensor_mul(s_y, s_y, tu)
            st[(B, 'sy')] = s_y
            while fill:
                fill.pop(0)()

        def qkv_block_fill(b):
            """Filler closures that compute block b's QKV/rms/rope."""
            fl = []
            fl += [qkv_seg(b, 0, s) for s in range(4)] + [extract_q(b, 0)]
            fl += [qkv_seg(b, 1, s) for s in range(4)] + [extract_q(b, 1)]
            fl += [qkv_seg(b, 2, s) for s in range(4)] + [extract_kv(b)]
            fl += [rms_mm(b), rms_newton(b)]
            fl += [qkv_seg(b, 3, s) for s in range(4)] + [extract_gate(b)]
            fl += [rk_unit(b)]
            fl += [vtr(b, j) for j in range(4)]
            fl += [rope_stage(b), rope_q(b, 0), rope_k(b), rope_q(b, 1)]
            return fl

        def interleave(a, bl):
            """Round-robin merge keeping each list's internal order."""
            out = []
            ia = ib = 0
            na, nbl = len(a), len(bl)
            tot = na + nbl
            for k in range(tot):
                # proportional progress
                if ia * nbl <= ib * na and ia < na:
                    out.append(a[ia]); ia += 1
                elif ib < nbl:
                    out.append(bl[ib]); ib += 1
                else:
                    out.append(a[ia]); ia += 1
            return out

        # ---------------- schedule
        load_w_x0()
        make_identity(nc, id64)
        make_upper_triangular(nc, tri, val=1.0, diag=True)
        load_tables(0)
        load_x(1)
        load_tables(1)
        nc.gpsimd.dma_start(out=Wo_sb, in_=Wo_d.ap().rearrange(
            "(cc p) h -> p cc h", p=128))
        # prologue: block 0 QKV, hc-major so the matmuls track the
        # streaming W/x chunk arrivals (4 accumulators in two sc2 tiles)
        psA = psum.tile([128, 2, 512], f32, tag="sc2", bufs=2, name="proA")
        psB = psum.tile([128, 2, 512], f32, tag="sc2", bufs=2, name="proB")
        for hc in range(HC):
            xt = xts[0][hc // 2][:, hc % 2, :]
            fst = (hc == 0)
            fin = (hc == HC - 1)
            nc.tensor.matmul(psA[:, 0, :], W_sb[:, hc, 0:128], xt,
                             start=fst, stop=fin)
            nc.tensor.matmul(psA[:, 1, :], W_sb[:, hc, 128:256], xt,
                             start=fst, stop=fin)
            nc.tensor.matmul(psB[:, 0, :], W_sb[:, hc, 256:384], xt,
                             start=fst, stop=fin)
            nc.tensor.matmul(psB[:, 1, :], W_sb[:, hc, 384:512], xt,
                             start=fst, stop=fin)
        for p in range(2):
            qr = wkp.tile([128, 512], bf16, tag=f"qr{p}", bufs=2,
                          name=f"qr0_{p}")
            nc.vector.tensor_copy(qr, psA[:, p, :])
            sq = wkp.tile([128, 512], bf16, tag=f"sq{p}", bufs=2,
                          name=f"sq0_{p}")
            nc.vector.tensor_mul(sq, qr, qr)
            st[(0, 'qr', p)] = qr
            st[(0, 'sq', p)] = sq
        kr = wkp.tile([64, 512], bf16, tag="kr", bufs=2, name="kr0")
        vr = wkp.tile([64, 512], bf16, tag="vr", bufs=2, name="vr0")
        nc.scalar.copy(kr, psB[0:64, 0, :])
        nc.scalar.copy(vr, psB[64:128, 0, :])
        ksq = wkp.tile([64, 512], bf16, tag="ksq", bufs=2, name="ksq0")
        nc.vector.tensor_mul(ksq, kr, kr)
        st[(0, 'kr')] = kr
        st[(0, 'vr')] = vr
        st[(0, 'ksq')] = ksq
        sig = wkp.tile([128, 512], f32, tag="sig", bufs=2, name="sig0")
        nc.scalar.activation(sig, psB[:, 1, :], AF.Exp, scale=-1.0)
        st[(0, 'sig')] = sig
        rms_mm(0)()
        rms_newton(0)()
        rk_unit(0)()
        for j in range(4):
            vtr(0, j)()
        rope_stage(0)()
        rope_q(0, 0)()
        rope_k(0)()
        rope_q(0, 1)()
        for B in range(NB):
            if B + 2 < NB:
                load_x(B + 2)
                load_tables(B + 2)
            last = (B == NB - 1)
            fill = []
            if B >= 1:
                fill += [finish_p(B - 1, 0), finish_p(B - 1, 1)]
            qf_fill = qkv_block_fill(B + 1) if B + 1 < NB else []
            op_fill = ([op_unit(B - 1, ss, half, alt=last, dve_drain=last)
                        for ss in range(4) for half in range(2)]
                       if B >= 1 else [])
            fill += interleave(qf_fill, op_fill)
            attention(B, fill)
        # tail
        finish_p(NB - 1, 0)()
        finish_p(NB - 1, 1)()
        for ss in range(4):
            for half in range(2):
                op_unit(NB - 1, ss, half, alt=True)()

    nc.compile()
    return nc


def _get_nc():
    if "nc" not in _BUILT:
        _BUILT["nc"] = _build_nc()
    return _BUILT["nc"]


# ---------------------------------------------------------------- entry point
def _install_ntff_hook():
    import types
    try:
        import antenv
        if "antenv.axon_hooks" in sys.modules:
            return True
        mod = types.ModuleType("antenv.axon_hooks")
        holder = [None]
        mod.set_axon_ntff_profile_hook = lambda h: holder.__setitem__(0, h)
        mod.get_axon_ntff_profile_hook = lambda: holder[0]
        sys.modules["antenv.axon_hooks"] = mod
        antenv.axon_hooks = mod
        from trn_agent_boot.trn_boot import _ntff_profile_via_ctypes
        hook = _ntff_profile_via_ctypes("/opt/axon/libaxon_pjrt.so")
        if hook is None:
            return False
        mod.set_axon_ntff_profile_hook(hook)
        return True
    except Exception:
        return False


def kernel(hidden_states, Wq, Wk, Wv, Wo, g_q, g_k):
    global LAST_EXEC_NS
    from concourse.bass_utils import run_bass_kernel_spmd

    in_maps = _host_prep(hidden_states, Wq, Wk, Wv, Wo, g_q, g_k)
    nc = _get_nc()
    trace = os.environ.get("KERNEL_TRACE", "0") == "1"
    if trace:
        trace = _install_ntff_hook()
    res = run_bass_kernel_spmd(nc, in_maps, list(range(NCORES)), trace=trace)
    LAST_EXEC_NS = res.exec_time_ns
    out = np.zeros((S, H), np.float32)
    for c in range(NCORES):
        out += np.asarray(res.results[c]["out"], np.float32)
    return out.reshape(1, S, H).astype(np.float32)
